# revision 33
# baseline (speedup 1.0000x reference)
"""Trainium2 Bass kernel for nn_Attention_cross (dual-branch cross-reuse attention).

Reference computation (B=4, N0=2048, C=768, H=12, hd=64, N=1024):
  x_diff, x_cond = x[:, :N], x[:, N:]
  q,k,v per branch = x @ w_qkv (per-head), attn = softmax(q k^T / sqrt(hd))
  o_d = ((attn_diff @ v_d) @ w_proj_diff + b_d) reused per-head with attn_cond
  o_c = (attn_cond @ v_c) @ w_proj_cond + b_c
  out = concat([o_d, o_c], axis=1)

Sharding: 8 cores = 4 batches x 2 head-groups (6 heads each). The
head-mixing projections are row-sharded with bf16 pair ReduceScatters
(2 chunks per branch) that overlap attention compute. Softmax
normalization is deferred through the linear ops: attention runs on
exp(scores) with row sums riding along as a ones-column in V; the
division is applied at the stores (per-partition scalar broadcasts
plus a small PE expander matmul for column groups). Attention matmul
inputs are bf16 (f32 PSUM accumulation); exp is batched 2 PSUM banks
per ACT instruction; PSUM->SBUF staging copies run on ACT during the
QKV stage where it is otherwise idle.
"""
import numpy as np

import concourse.bass as bass
import concourse.tile as tile
from concourse import bacc, mybir
from concourse.bass_utils import run_bass_kernel_spmd
from concourse.masks import make_identity
from concourse.alu_op_type import AluOpType

F32 = mybir.dt.float32
F32R = mybir.dt.float32r
BF16 = mybir.dt.bfloat16
Exp = mybir.ActivationFunctionType.Exp
Copy = mybir.ActivationFunctionType.Copy

B, N0, C = 4, 2048, 768
H, HD = 12, 64
N = N0 // 2              # 1024 sequence per branch
HPC = H // 2             # 6 heads per core
CW = HPC * HD            # 384 own C-columns/rows
NCH = N // 128           # 8 chunks of 128 along n/m
CCH = C // 128           # 6 chunks of 128 along C
NB = N // 512            # 2 blocks of 512 along n
NH = N // 2              # 512 rows per RS chunk
SCALE = HD ** -0.5
LAG = 6                  # cond units before the first deferred o2 unit

N_CORES = 8
GROUPS = [[0, 1], [2, 3], [4, 5], [6, 7]]

_CACHE = {}


def _build():
    nc = bacc.Bacc("TRN2", target_bir_lowering=False, debug=False,
                   num_devices=N_CORES)

    x_b = nc.dram_tensor("x_b", [N0, C], BF16, kind="ExternalInput").ap()
    wqk_d = nc.dram_tensor("wqk_d", [C, 2 * CW], BF16, kind="ExternalInput").ap()
    wqk_c = nc.dram_tensor("wqk_c", [C, 2 * CW], BF16, kind="ExternalInput").ap()
    wv_d = nc.dram_tensor("wv_d", [C, CW], BF16, kind="ExternalInput").ap()
    wv_c = nc.dram_tensor("wv_c", [C, CW], BF16, kind="ExternalInput").ap()
    wp_d = nc.dram_tensor("wp_d", [CW, C], BF16, kind="ExternalInput").ap()
    wp_c = nc.dram_tensor("wp_c", [CW, C], BF16, kind="ExternalInput").ap()
    bias_d = nc.dram_tensor("bias_d", [1, CW], F32, kind="ExternalInput").ap()
    bias_c = nc.dram_tensor("bias_c", [1, CW], F32, kind="ExternalInput").ap()
    # o_d stored transposed per head: [HPC, HD, N]; host re-layouts.
    o_d_t = nc.dram_tensor("o_d_t", [HPC, HD, N], F32, kind="ExternalOutput").ap()
    o_c_full = nc.dram_tensor("o_c_cols", [N, CW], F32, kind="ExternalOutput").ap()

    # ReduceScatter halves (bf16): input [2, NH, CW] slot-major; each pair
    # member receives its slot reduced: out [NH, CW].
    cc_in_d = [nc.dram_tensor(f"cc_in_d{k}", [2, NH, CW], BF16).ap() for k in range(2)]
    cc_out_d = [nc.dram_tensor(f"cc_out_d{k}", [NH, CW], BF16).ap() for k in range(2)]
    cc_in_c = [nc.dram_tensor(f"cc_in_c{k}", [2, NH, CW], BF16).ap() for k in range(2)]
    cc_out_c = [nc.dram_tensor(f"cc_out_c{k}", [NH, CW], BF16).ap() for k in range(2)]

    with tile.TileContext(nc) as tc:
        _body(nc, tc, x_b, wqk_d, wqk_c, wv_d, wv_c, wp_d, wp_c,
              bias_d, bias_c, o_d_t, o_c_full,
              cc_in_d, cc_out_d, cc_in_c, cc_out_c)
    nc.compile()
    return nc


def _body(nc, tc, x_b, wqk_d, wqk_c, wv_d, wv_c, wp_d, wp_c,
          bias_d, bias_c, o_d_t, o_c_full,
          cc_in_d, cc_out_d, cc_in_c, cc_out_c):
    from contextlib import ExitStack
    ctx = ExitStack()
    with ctx:
        ctx.enter_context(nc.allow_low_precision(reason="bf16 matmul inputs by design"))
        singles = ctx.enter_context(tc.tile_pool(name="singles", bufs=1))
        ident = singles.tile([128, 128], F32)
        make_identity(nc, ident[:])
        identb = singles.tile([128, 128], BF16)
        nc.vector.tensor_copy(identb[:], ident[:])
        ones_b = singles.tile([1, HD], BF16)
        nc.vector.memset(ones_b[:], 1.0)

        big = ctx.enter_context(tc.tile_pool(name="big", bufs=1))
        qkT = {}    # branch -> [128, 6, N] bf16  rows: [q h0..h5 | k h0..h5]
        v_aug = {}  # branch -> [128, HPC, NCH, 65] bf16 (ones col -> row sums)
        u_t = {}    # branch -> [128, 3, N] bf16 (UNNORMALIZED (expS@v)^T)
        # reciprocal row sums, partition 0 only, free-indexed by unit
        # u = h*NB+nb (partition_broadcast and engine slicing need base 0)
        rT = {}     # branch -> [128, HPC*NB, 512] bf16 (partition 0 used)
        for br in ("d", "c"):
            qkT[br] = big.tile([128, 2 * CW // 128, N], BF16, tag=f"qkT_{br}", name=f"qkT_{br}")
            v_aug[br] = big.tile([128, HPC, NCH, HD + 1], BF16, tag=f"v_{br}", name=f"v_{br}")
            nc.vector.memset(v_aug[br][:, :, :, HD:HD + 1], 1.0)
            u_t[br] = big.tile([128, CW // 128, N], BF16, tag=f"u_{br}", name=f"u_{br}")
            rT[br] = big.tile([128, HPC * NB, 512], BF16, tag=f"rT_{br}", name=f"rT_{br}")
        p_d = big.tile([128, NCH, CW], BF16, tag="p_d")  # normalized+biased proj_diff

        # qkv weights: one consolidated DMA per tensor
        wqk_r = {}
        wv_r = {}
        wp_r = {}
        wsrc = {"d": (wqk_d, wv_d), "c": (wqk_c, wv_c)}
        for br in ("d", "c"):
            wqk_r[br] = big.tile([128, CCH, 2 * CW], BF16, tag=f"wqk_{br}", name=f"wqk_{br}")
            wv_r[br] = big.tile([128, CCH, CW], BF16, tag=f"wv_{br}", name=f"wv_{br}")

        # ---------- Stage A: x load + transpose + QKV for both branches ----
        with tc.tile_pool(name="xt_pool", bufs=1) as xt_pool, \
             tc.tile_pool(name="ab", bufs=3) as ab, \
             tc.tile_pool(name="ps_tp", bufs=1, space="PSUM") as ps_tp, \
             tc.tile_pool(name="ps_ab", bufs=2, space="PSUM") as ps_ab:
            for bi, br in enumerate(("d", "c")):
                half = bi * N
                wqk, wv = wsrc[br]
                nc.sync.dma_start(
                    out=wqk_r[br][:],
                    in_=wqk.rearrange("(i p) f -> p i f", p=128))
                nc.sync.dma_start(
                    out=wv_r[br][:],
                    in_=wv.rearrange("(i p) f -> p i f", p=128))
                xT = xt_pool.tile([128, CCH, N], BF16, tag="xT", bufs=2, name=f"xT_{br}")
                for jg in range(2):      # n-groups of 4 chunks
                    tp_tiles = {}
                    for ci in range(CCH):
                        tp_tiles[ci] = ps_tp.tile([128, 512], BF16, tag=f"tp{ci}",
                                                  name=f"tp_{br}_{jg}_{ci}")
                    xn = ab.tile([128, 4, C], BF16, tag="x_nat", bufs=2)
                    nc.sync.dma_start(
                        out=xn[:],
                        in_=x_b[half + jg * 512: half + (jg + 1) * 512, :]
                        .rearrange("(j p) c -> p j c", p=128))
                    for jj in range(4):
                        for ci in range(CCH):
                            nc.tensor.transpose(
                                tp_tiles[ci][:, jj * 128:(jj + 1) * 128],
                                xn[:, jj, ci * 128:(ci + 1) * 128], identb[:])
                    for ci in range(CCH):
                        nc.scalar.activation(
                            xT[:, ci, jg * 512:(jg + 1) * 512], tp_tiles[ci][:], Copy)

                for fi in range(2 * CW // 128):
                    for nb in range(NB):
                        ps = ps_ab.tile([128, 512], F32, tag="qk_ps")
                        for ci in range(CCH):
                            nc.tensor.matmul(
                                ps[:],
                                wqk_r[br][:, ci, fi * 128:(fi + 1) * 128],
                                xT[:, ci, nb * 512:(nb + 1) * 512],
                                start=(ci == 0), stop=(ci == CCH - 1))
                        nc.scalar.activation(
                            qkT[br][:, fi, nb * 512:(nb + 1) * 512], ps[:], Copy)
                for mch in range(NCH):
                    ps = ps_ab.tile([128, 512], F32, tag="qk_ps")
                    for ci in range(CCH):
                        nc.tensor.matmul(
                            ps[:, 0:CW], xT[:, ci, mch * 128:(mch + 1) * 128],
                            wv_r[br][:, ci, :],
                            start=(ci == 0), stop=(ci == CCH - 1))
                    nc.scalar.activation(
                        v_aug[br][:, :, mch, 0:HD],
                        ps[:, 0:CW].rearrange("p (h d) -> p h d", h=HPC), Copy)

        # proj weights + bias: needed from mid stage C on
        for br, wp in (("d", wp_d), ("c", wp_c)):
            wp_r[br] = big.tile([128, CW // 128, C], BF16, tag=f"wp_{br}", name=f"wp_{br}")
            nc.sync.dma_start(out=wp_r[br][:],
                              in_=wp.rearrange("(i p) f -> p i f", p=128))
        bias_bd = big.tile([128, CW], F32, tag="bias_bd")
        nc.sync.dma_start(out=bias_bd[:], in_=bias_d.to_broadcast([128, CW]))
        bias_bc = big.tile([128, CW], F32, tag="bias_bc")
        nc.sync.dma_start(out=bias_bc[:], in_=bias_c.to_broadcast([128, CW]))

        # ---------- attention unit helpers ----------
        def emit_scores(br, h, nb, eT_pool, ps_sc):
            qc, qo = divmod(h * HD, 128)
            kc, ko = divmod(CW + h * HD, 128)
            eT = eT_pool.tile([128, NCH, 512], BF16, tag="eT", bufs=LAG + 2,
                              name=f"eT_{br}_{h}_{nb}")
            for pr in range(NCH // 2):
                ps = ps_sc.tile([128, 1024], F32, tag="sc_ps")
                for k in range(2):
                    mch = pr * 2 + k
                    nc.tensor.matmul(
                        ps[:, k * 512:(k + 1) * 512],
                        qkT[br][ko:ko + HD, kc, mch * 128:(mch + 1) * 128],
                        qkT[br][qo:qo + HD, qc, nb * 512:(nb + 1) * 512],
                        start=True, stop=True)
                nc.scalar.activation(
                    eT[:, pr * 2:pr * 2 + 2, :].rearrange("p m q -> p (m q)"),
                    ps[:], Exp)
            return eT

        def emit_avmm(br, h, nb, eT, ps_av):
            # AV accumulate (u rows 0..63, row sums at 64) + reciprocal
            ps_u = ps_av.tile([128, 512], F32, tag="av_ps")
            for mch in range(NCH):
                nc.tensor.matmul(
                    ps_u[0:HD + 1, :], v_aug[br][:, h, mch, :], eT[:, mch, :],
                    start=(mch == 0), stop=(mch == NCH - 1))
            nc.vector.reciprocal(rT[br][0:1, h * NB + nb, :], ps_u[HD:HD + 1, :])
            return ps_u

        def emit_fin(br, h, nb, ps_u):
            # broadcast 1/s across 64 rows (K=1 PE matmul into rows 64..127
            # of the same bank), then normalize u into SBUF
            uc, uo = divmod(h * HD, 128)
            nc.tensor.matmul(
                ps_u[HD:2 * HD, :], ones_b[:],
                rT[br][0:1, h * NB + nb, :], start=True, stop=True)
            dst = u_t[br][uo:uo + HD, uc, nb * 512:(nb + 1) * 512]
            nc.vector.tensor_copy(dst, ps_u[0:HD, :])
            nc.vector.tensor_mul(dst, dst, ps_u[HD:2 * HD, :])

        def proj_half(u, wp, cc_in_half, half, pj, ps_pj):
            for nch in range(half * 4, half * 4 + 4):
                st = pj.tile([128, 2, CW], BF16, tag="pj_st")
                for slot in range(2):
                    ps = ps_pj.tile([128, CW], F32, tag="pj_ps")
                    for ci in range(CW // 128):
                        nc.tensor.matmul(
                            ps[:],
                            u[:, ci, nch * 128:(nch + 1) * 128],
                            wp[:, ci, slot * CW:(slot + 1) * CW],
                            start=(ci == 0), stop=(ci == CW // 128 - 1))
                    nc.scalar.activation(st[:, slot, :], ps[:], Copy)
                r0 = (nch - half * 4) * 128
                nc.sync.dma_start(
                    out=cc_in_half[:, r0:r0 + 128, :].rearrange("s p f -> p s f"),
                    in_=st[:])

        with tc.tile_pool(name="eT_pool", bufs=2) as eT_pool, \
             tc.tile_pool(name="pj", bufs=2) as pj, \
             tc.tile_pool(name="ps_sc", bufs=2, space="PSUM") as ps_sc, \
             tc.tile_pool(name="ps_av", bufs=2, space="PSUM") as ps_av, \
             tc.tile_pool(name="ps_pj", bufs=1, space="PSUM") as ps_pj:

            units = [(h, nb) for nb in range(NB) for h in range(HPC)]

            def rs(cc_in, cc_out):
                nc.gpsimd.collective_compute(
                    "ReduceScatter", AluOpType.add, replica_groups=GROUPS,
                    ins=[cc_in], outs=[cc_out])

            cc_in = {"d": cc_in_d, "c": cc_in_c}
            cc_out = {"d": cc_out_d, "c": cc_out_c}

            # ---------- Stages C+E: one software-pipelined loop ----------
            with tc.tile_pool(name="o2pool", bufs=2) as o2p, \
                 tc.tile_pool(name="pdl", bufs=2) as pdl, \
                 tc.tile_pool(name="ps_o2", bufs=1, space="PSUM") as ps_o2:

                def pd_load(mch, pin_ap):
                    # u was pre-normalized; p_d[mch] = pin + bias_d
                    nc.vector.tensor_add(p_d[:, mch, :], pin_ap, bias_bd[:])

                def emit_o2(h, nb, eT, ps_pool):
                    # o2 = (attn_cond @ p_d) per head, normalized by r_c
                    # (K=1 broadcast into rows 64..127); stored transposed
                    ps_o = ps_pool.tile([128, 512], F32,
                                        tag="o2_ps" if ps_pool is ps_o2 else "av_ps")
                    for mch in range(NCH):
                        nc.tensor.matmul(
                            ps_o[0:HD, :], p_d[:, mch, h * HD:(h + 1) * HD],
                            eT[:, mch, :],
                            start=(mch == 0), stop=(mch == NCH - 1))
                    nc.tensor.matmul(
                        ps_o[HD:2 * HD, :], ones_b[:],
                        rT["c"][0:1, h * NB + nb, :], start=True, stop=True)
                    o2T = o2p.tile([HD, 512], F32, tag="o2T")
                    nc.vector.tensor_copy(o2T[:], ps_o[0:HD, :])
                    nc.vector.tensor_mul(o2T[:], o2T[:], ps_o[HD:2 * HD, :])
                    nc.sync.dma_start(
                        out=o_d_t[h, :, nb * 512:(nb + 1) * 512], in_=o2T[:])

                all_units = [("d", h, nb) for h, nb in units] + \
                            [("c", h, nb) for h, nb in units]
                avq = []
                finq = []
                o2q = []
                ci_ = 0   # cond unit counter
                for br, h, nb in all_units:
                    eT = emit_scores(br, h, nb, eT_pool, ps_sc)
                    if avq:
                        pbr, ph, pnb, peT = avq.pop(0)
                        finq.append((pbr, ph, pnb,
                                     emit_avmm(pbr, ph, pnb, peT, ps_av)))
                    if len(finq) > 1:
                        entry = finq.pop(0)
                        fbr, fh, fnb, fps = entry
                        emit_fin(fbr, fh, fnb, fps)
                        if fh == HPC - 1:
                            proj_half(u_t[fbr], wp_r[fbr], cc_in[fbr][fnb],
                                      fnb, pj, ps_pj)
                            rs(cc_in[fbr][fnb], cc_out[fbr][fnb])
                    if br == "c":
                        if ci_ == LAG:
                            for k in range(2):
                                pin = pdl.tile([128, 4, CW], BF16, tag="p_in",
                                               bufs=2, name=f"pin_{k}")
                                nc.sync.dma_start(
                                    out=pin[:],
                                    in_=cc_out_d[k].rearrange(
                                        "(j p) f -> p j f", p=128))
                                for j in range(4):
                                    pd_load(k * 4 + j, pin[:, j, :])
                        if ci_ > LAG:
                            uh, unb, ueT = o2q.pop(0)
                            emit_o2(uh, unb, ueT, ps_o2)
                        o2q.append((h, nb, eT))
                        ci_ += 1
                    avq.append((br, h, nb, eT))
                # drain attention pipeline
                pbr, ph, pnb, peT = avq.pop(0)
                finq.append((pbr, ph, pnb, emit_avmm(pbr, ph, pnb, peT, ps_av)))
                while finq:
                    fbr, fh, fnb, fps = finq.pop(0)
                    emit_fin(fbr, fh, fnb, fps)
                    if fh == HPC - 1:
                        proj_half(u_t[fbr], wp_r[fbr], cc_in[fbr][fnb],
                                  fnb, pj, ps_pj)
                        rs(cc_in[fbr][fnb], cc_out[fbr][fnb])

                # o_c stores: half0 (RS_c0 landed during stage E), then the
                # deferred o2 tail (pure PE) overlapping RS_c1, then half1.
                def oc_store(k, ocs):
                    pin = ocs.tile([128, 4, CW], BF16, tag="pc_in")
                    nc.sync.dma_start(
                        out=pin[:],
                        in_=cc_out_c[k].rearrange("(j p) f -> p j f", p=128))
                    ob = ocs.tile([128, 4, CW], F32, tag="oc_out")
                    for j in range(4):
                        nc.vector.tensor_add(ob[:, j, :], pin[:, j, :], bias_bc[:])
                    nc.sync.dma_start(
                        out=o_c_full[k * 512:(k + 1) * 512, :]
                        .rearrange("(j p) f -> p j f", p=128),
                        in_=ob[:])

                with tc.tile_pool(name="oc_pool", bufs=1) as ocs:
                    oc_store(0, ocs)
                    alt = 0
                    while o2q:
                        uh, unb, ueT = o2q.pop(0)
                        emit_o2(uh, unb, ueT, ps_av if alt % 2 else ps_o2)
                        alt += 1
                    oc_store(1, ocs)


def _prep_inputs(x, w_qkv_diff, w_qkv_cond, w_proj_diff, b_proj_diff,
                 w_proj_cond, b_proj_cond):
    import ml_dtypes
    bf = ml_dtypes.bfloat16
    in_maps = []
    for c in range(N_CORES):
        b, hg = divmod(c, 2)
        s = slice(hg * CW, (hg + 1) * CW)
        sk = slice(C + hg * CW, C + (hg + 1) * CW)
        sv = slice(2 * C + hg * CW, 2 * C + (hg + 1) * CW)
        m = {
            "x_b": np.ascontiguousarray(x[b].astype(bf)),
            "wqk_d": np.ascontiguousarray(np.concatenate(
                [w_qkv_diff[:, s] * SCALE, w_qkv_diff[:, sk]], axis=1).astype(bf)),
            "wqk_c": np.ascontiguousarray(np.concatenate(
                [w_qkv_cond[:, s] * SCALE, w_qkv_cond[:, sk]], axis=1).astype(bf)),
            "wv_d": np.ascontiguousarray(w_qkv_diff[:, sv].astype(bf)),
            "wv_c": np.ascontiguousarray(w_qkv_cond[:, sv].astype(bf)),
            "wp_d": np.ascontiguousarray(w_proj_diff[s, :].astype(bf)),
            "wp_c": np.ascontiguousarray(w_proj_cond[s, :].astype(bf)),
            "bias_d": np.ascontiguousarray(b_proj_diff[None, s], np.float32),
            "bias_c": np.ascontiguousarray(b_proj_cond[None, s], np.float32),
        }
        in_maps.append(m)
    return in_maps


def kernel(x, w_qkv_diff, w_qkv_cond, w_proj_diff, b_proj_diff,
           w_proj_cond, b_proj_cond):
    x = np.asarray(x)
    w_qkv_diff = np.asarray(w_qkv_diff)
    w_qkv_cond = np.asarray(w_qkv_cond)
    w_proj_diff = np.asarray(w_proj_diff)
    b_proj_diff = np.asarray(b_proj_diff)
    w_proj_cond = np.asarray(w_proj_cond)
    b_proj_cond = np.asarray(b_proj_cond)

    if "nc" not in _CACHE:
        _CACHE["nc"] = _build()
    nc = _CACHE["nc"]
    in_maps = _prep_inputs(x, w_qkv_diff, w_qkv_cond, w_proj_diff,
                           b_proj_diff, w_proj_cond, b_proj_cond)
    res = run_bass_kernel_spmd(nc, in_maps, list(range(N_CORES))).results

    o_d = np.empty((B, N, C), np.float32)
    o_c = np.empty((B, N, C), np.float32)
    for c in range(N_CORES):
        b, hg = divmod(c, 2)
        odt = np.asarray(res[c]["o_d_t"], np.float32)   # [HPC, HD, N]
        o_d[b][:, hg * CW:(hg + 1) * CW] = odt.transpose(2, 0, 1).reshape(N, CW)
        o_c[b][:, hg * CW:(hg + 1) * CW] = res[c]["o_c_cols"]
    return np.concatenate([o_d, o_c], axis=1)


# revision 35
# speedup vs baseline: 1.0170x; 1.0170x over previous
"""Trainium2 Bass kernel for nn_Attention_cross (dual-branch cross-reuse attention).

Reference computation (B=4, N0=2048, C=768, H=12, hd=64, N=1024):
  x_diff, x_cond = x[:, :N], x[:, N:]
  q,k,v per branch = x @ w_qkv (per-head), attn = softmax(q k^T / sqrt(hd))
  o_d = ((attn_diff @ v_d) @ w_proj_diff + b_d) reused per-head with attn_cond
  o_c = (attn_cond @ v_c) @ w_proj_cond + b_c
  out = concat([o_d, o_c], axis=1)

Sharding: 8 cores = 4 batches x 2 head-groups (6 heads each). The
head-mixing projections are row-sharded with bf16 pair ReduceScatters
(2 chunks per branch) that overlap attention compute. Softmax
normalization is deferred through the linear ops: attention runs on
exp(scores) with row sums riding along as a ones-column in V; the
division is applied at the stores (per-partition scalar broadcasts
plus a small PE expander matmul for column groups). Attention matmul
inputs are bf16 (f32 PSUM accumulation); exp is batched 2 PSUM banks
per ACT instruction; PSUM->SBUF staging copies run on ACT during the
QKV stage where it is otherwise idle.
"""
import numpy as np

import concourse.bass as bass
import concourse.tile as tile
from concourse import bacc, mybir
from concourse.bass_utils import run_bass_kernel_spmd
from concourse.masks import make_identity
from concourse.alu_op_type import AluOpType

F32 = mybir.dt.float32
F32R = mybir.dt.float32r
BF16 = mybir.dt.bfloat16
Exp = mybir.ActivationFunctionType.Exp
Copy = mybir.ActivationFunctionType.Copy

B, N0, C = 4, 2048, 768
H, HD = 12, 64
N = N0 // 2              # 1024 sequence per branch
HPC = H // 2             # 6 heads per core
CW = HPC * HD            # 384 own C-columns/rows
NCH = N // 128           # 8 chunks of 128 along n/m
CCH = C // 128           # 6 chunks of 128 along C
NB = N // 512            # 2 blocks of 512 along n
NH = N // 2              # 512 rows per RS chunk
SCALE = HD ** -0.5
LAG = 6                  # cond units before the first deferred o2 unit

N_CORES = 8
GROUPS = [[0, 1], [2, 3], [4, 5], [6, 7]]

_CACHE = {}


def _build():
    nc = bacc.Bacc("TRN2", target_bir_lowering=False, debug=False,
                   num_devices=N_CORES)

    x_b = nc.dram_tensor("x_b", [N0, C], BF16, kind="ExternalInput").ap()
    wqk_d = nc.dram_tensor("wqk_d", [C, 2 * CW], BF16, kind="ExternalInput").ap()
    wqk_c = nc.dram_tensor("wqk_c", [C, 2 * CW], BF16, kind="ExternalInput").ap()
    wv_d = nc.dram_tensor("wv_d", [C, CW], BF16, kind="ExternalInput").ap()
    wv_c = nc.dram_tensor("wv_c", [C, CW], BF16, kind="ExternalInput").ap()
    wp_d = nc.dram_tensor("wp_d", [CW, C], BF16, kind="ExternalInput").ap()
    wp_c = nc.dram_tensor("wp_c", [CW, C], BF16, kind="ExternalInput").ap()
    bias_d = nc.dram_tensor("bias_d", [1, CW], F32, kind="ExternalInput").ap()
    bias_c = nc.dram_tensor("bias_c", [1, CW], F32, kind="ExternalInput").ap()
    # o_d stored transposed per head: [HPC, HD, N]; host re-layouts.
    o_d_t = nc.dram_tensor("o_d_t", [HPC, HD, N], F32, kind="ExternalOutput").ap()
    o_c_full = nc.dram_tensor("o_c_cols", [N, CW], F32, kind="ExternalOutput").ap()

    # ReduceScatter halves (bf16): input [2, NH, CW] slot-major; each pair
    # member receives its slot reduced: out [NH, CW].
    cc_in_d = [nc.dram_tensor(f"cc_in_d{k}", [2, NH, CW], BF16).ap() for k in range(2)]
    cc_out_d = [nc.dram_tensor(f"cc_out_d{k}", [NH, CW], BF16).ap() for k in range(2)]
    cc_in_c = [nc.dram_tensor(f"cc_in_c{k}", [2, NH, CW], BF16).ap() for k in range(2)]
    cc_out_c = [nc.dram_tensor(f"cc_out_c{k}", [NH, CW], BF16).ap() for k in range(2)]

    with tile.TileContext(nc) as tc:
        _body(nc, tc, x_b, wqk_d, wqk_c, wv_d, wv_c, wp_d, wp_c,
              bias_d, bias_c, o_d_t, o_c_full,
              cc_in_d, cc_out_d, cc_in_c, cc_out_c)
    nc.compile()
    return nc


def _body(nc, tc, x_b, wqk_d, wqk_c, wv_d, wv_c, wp_d, wp_c,
          bias_d, bias_c, o_d_t, o_c_full,
          cc_in_d, cc_out_d, cc_in_c, cc_out_c):
    from contextlib import ExitStack
    ctx = ExitStack()
    with ctx:
        ctx.enter_context(nc.allow_low_precision(reason="bf16 matmul inputs by design"))
        singles = ctx.enter_context(tc.tile_pool(name="singles", bufs=1))
        ident = singles.tile([128, 128], F32)
        make_identity(nc, ident[:])
        identb = singles.tile([128, 128], BF16)
        nc.vector.tensor_copy(identb[:], ident[:])
        ones_b = singles.tile([1, HD], BF16)
        nc.vector.memset(ones_b[:], 1.0)

        big = ctx.enter_context(tc.tile_pool(name="big", bufs=1))
        qkT = {}    # branch -> [128, 6, N] bf16  rows: [q h0..h5 | k h0..h5]
        v_aug = {}  # branch -> [128, HPC, NCH, 65] bf16 (ones col -> row sums)
        u_t = {}    # branch -> [128, 3, N] bf16 (UNNORMALIZED (expS@v)^T)
        # reciprocal row sums, partition 0 only, free-indexed by unit
        # u = h*NB+nb (partition_broadcast and engine slicing need base 0)
        rT = {}     # branch -> [128, HPC*NB, 512] bf16 (partition 0 used)
        for br in ("d", "c"):
            qkT[br] = big.tile([128, 2 * CW // 128, N], BF16, tag=f"qkT_{br}", name=f"qkT_{br}")
            v_aug[br] = big.tile([128, HPC, NCH, HD + 1], BF16, tag=f"v_{br}", name=f"v_{br}")
            nc.vector.memset(v_aug[br][:, :, :, HD:HD + 1], 1.0)
            u_t[br] = big.tile([128, CW // 128, N], BF16, tag=f"u_{br}", name=f"u_{br}")
            rT[br] = big.tile([128, HPC * NB, 512], BF16, tag=f"rT_{br}", name=f"rT_{br}")
        p_d = big.tile([128, NCH, CW], BF16, tag="p_d")  # normalized+biased proj_diff

        # qkv weights: one consolidated DMA per tensor
        wqk_r = {}
        wv_r = {}
        wp_r = {}
        wsrc = {"d": (wqk_d, wv_d), "c": (wqk_c, wv_c)}
        for br in ("d", "c"):
            wqk_r[br] = big.tile([128, CCH, 2 * CW], BF16, tag=f"wqk_{br}", name=f"wqk_{br}")
            wv_r[br] = big.tile([128, CCH, CW], BF16, tag=f"wv_{br}", name=f"wv_{br}")

        # ---------- Stage A: x load + transpose + QKV for both branches ----
        with tc.tile_pool(name="xt_pool", bufs=1) as xt_pool, \
             tc.tile_pool(name="ab", bufs=3) as ab, \
             tc.tile_pool(name="ps_tp", bufs=1, space="PSUM") as ps_tp, \
             tc.tile_pool(name="ps_ab", bufs=2, space="PSUM") as ps_ab:
            for bi, br in enumerate(("d", "c")):
                half = bi * N
                wqk, wv = wsrc[br]
                nc.sync.dma_start(
                    out=wqk_r[br][:],
                    in_=wqk.rearrange("(i p) f -> p i f", p=128))
                nc.sync.dma_start(
                    out=wv_r[br][:],
                    in_=wv.rearrange("(i p) f -> p i f", p=128))
                xT = xt_pool.tile([128, CCH, N], BF16, tag="xT", bufs=2, name=f"xT_{br}")
                for jg in range(2):      # n-groups of 4 chunks
                    tp_tiles = {}
                    for ci in range(CCH):
                        tp_tiles[ci] = ps_tp.tile([128, 512], BF16, tag=f"tp{ci}",
                                                  name=f"tp_{br}_{jg}_{ci}")
                    xn = ab.tile([128, 4, C], BF16, tag="x_nat", bufs=2)
                    nc.sync.dma_start(
                        out=xn[:],
                        in_=x_b[half + jg * 512: half + (jg + 1) * 512, :]
                        .rearrange("(j p) c -> p j c", p=128))
                    for jj in range(4):
                        for ci in range(CCH):
                            nc.tensor.transpose(
                                tp_tiles[ci][:, jj * 128:(jj + 1) * 128],
                                xn[:, jj, ci * 128:(ci + 1) * 128], identb[:])
                    for ci in range(CCH):
                        nc.scalar.activation(
                            xT[:, ci, jg * 512:(jg + 1) * 512], tp_tiles[ci][:], Copy)

                for fi in range(2 * CW // 128):
                    for nb in range(NB):
                        ps = ps_ab.tile([128, 512], F32, tag="qk_ps")
                        for ci in range(CCH):
                            nc.tensor.matmul(
                                ps[:],
                                wqk_r[br][:, ci, fi * 128:(fi + 1) * 128],
                                xT[:, ci, nb * 512:(nb + 1) * 512],
                                start=(ci == 0), stop=(ci == CCH - 1))
                        nc.scalar.activation(
                            qkT[br][:, fi, nb * 512:(nb + 1) * 512], ps[:], Copy)
                for mch in range(NCH):
                    ps = ps_ab.tile([128, 512], F32, tag="qk_ps")
                    for ci in range(CCH):
                        nc.tensor.matmul(
                            ps[:, 0:CW], xT[:, ci, mch * 128:(mch + 1) * 128],
                            wv_r[br][:, ci, :],
                            start=(ci == 0), stop=(ci == CCH - 1))
                    nc.scalar.activation(
                        v_aug[br][:, :, mch, 0:HD],
                        ps[:, 0:CW].rearrange("p (h d) -> p h d", h=HPC), Copy)

        # proj weights + bias: needed from mid stage C on
        for br, wp in (("d", wp_d), ("c", wp_c)):
            wp_r[br] = big.tile([128, CW // 128, C], BF16, tag=f"wp_{br}", name=f"wp_{br}")
            nc.sync.dma_start(out=wp_r[br][:],
                              in_=wp.rearrange("(i p) f -> p i f", p=128))
        bias_bd = big.tile([128, CW], F32, tag="bias_bd")
        nc.sync.dma_start(out=bias_bd[:], in_=bias_d.to_broadcast([128, CW]))
        bias_bc = big.tile([128, CW], F32, tag="bias_bc")
        nc.sync.dma_start(out=bias_bc[:], in_=bias_c.to_broadcast([128, CW]))

        # ---------- attention unit helpers ----------
        def emit_scores(br, h, nb, eT_pool, ps_sc):
            qc, qo = divmod(h * HD, 128)
            kc, ko = divmod(CW + h * HD, 128)
            eT = eT_pool.tile([128, NCH, 512], BF16, tag="eT", bufs=LAG + 2,
                              name=f"eT_{br}_{h}_{nb}")
            for pr in range(NCH // 2):
                ps = ps_sc.tile([128, 1024], F32, tag="sc_ps")
                for k in range(2):
                    mch = pr * 2 + k
                    nc.tensor.matmul(
                        ps[:, k * 512:(k + 1) * 512],
                        qkT[br][ko:ko + HD, kc, mch * 128:(mch + 1) * 128],
                        qkT[br][qo:qo + HD, qc, nb * 512:(nb + 1) * 512],
                        start=True, stop=True)
                nc.scalar.activation(
                    eT[:, pr * 2:pr * 2 + 2, :].rearrange("p m q -> p (m q)"),
                    ps[:], Exp)
            return eT

        def emit_avmm(br, h, nb, eT, ps_av):
            # AV accumulate (u rows 0..63, row sums at 64) + reciprocal
            ps_u = ps_av.tile([128, 512], F32, tag="av_ps")
            for mch in range(NCH):
                nc.tensor.matmul(
                    ps_u[0:HD + 1, :], v_aug[br][:, h, mch, :], eT[:, mch, :],
                    start=(mch == 0), stop=(mch == NCH - 1))
            nc.vector.reciprocal(rT[br][0:1, h * NB + nb, :], ps_u[HD:HD + 1, :])
            return ps_u

        def emit_fin(br, h, nb, ps_u):
            # broadcast 1/s across 64 rows (K=1 PE matmul into rows 64..127
            # of the same bank), then normalize u into SBUF
            uc, uo = divmod(h * HD, 128)
            nc.tensor.matmul(
                ps_u[HD:2 * HD, :], ones_b[:],
                rT[br][0:1, h * NB + nb, :], start=True, stop=True)
            dst = u_t[br][uo:uo + HD, uc, nb * 512:(nb + 1) * 512]
            nc.vector.tensor_copy(dst, ps_u[0:HD, :])
            nc.vector.tensor_mul(dst, dst, ps_u[HD:2 * HD, :])

        def proj_half(u, wp, cc_in_half, half, pj, ps_pj):
            for nch in range(half * 4, half * 4 + 4):
                st = pj.tile([128, 2, CW], BF16, tag="pj_st")
                for slot in range(2):
                    ps = ps_pj.tile([128, CW], F32, tag="pj_ps")
                    for ci in range(CW // 128):
                        nc.tensor.matmul(
                            ps[:],
                            u[:, ci, nch * 128:(nch + 1) * 128],
                            wp[:, ci, slot * CW:(slot + 1) * CW],
                            start=(ci == 0), stop=(ci == CW // 128 - 1))
                    nc.scalar.activation(st[:, slot, :], ps[:], Copy)
                r0 = (nch - half * 4) * 128
                nc.sync.dma_start(
                    out=cc_in_half[:, r0:r0 + 128, :].rearrange("s p f -> p s f"),
                    in_=st[:])

        with tc.tile_pool(name="eT_pool", bufs=2) as eT_pool, \
             tc.tile_pool(name="pj", bufs=2) as pj, \
             tc.tile_pool(name="ps_sc", bufs=2, space="PSUM") as ps_sc, \
             tc.tile_pool(name="ps_av", bufs=2, space="PSUM") as ps_av, \
             tc.tile_pool(name="ps_pj", bufs=1, space="PSUM") as ps_pj:

            units = [(h, nb) for nb in range(NB) for h in range(HPC)]

            def rs(cc_in_ap, cc_out_ap):
                nc.gpsimd.collective_compute(
                    "ReduceScatter", AluOpType.add, replica_groups=GROUPS,
                    ins=[cc_in_ap], outs=[cc_out_ap])

            cc_in = {"d": cc_in_d, "c": cc_in_c}
            cc_out = {"d": cc_out_d, "c": cc_out_c}

            def fin_and_proj(br, finq):
                fh, fnb, fps = finq.pop(0)
                emit_fin(br, fh, fnb, fps)
                if fh == HPC - 1:
                    proj_half(u_t[br], wp_r[br], cc_in[br][fnb], fnb, pj, ps_pj)
                    rs(cc_in[br][fnb], cc_out[br][fnb])

            # ---------- Stage C: diff attention, proj + chunked RS ----------
            avq = []
            finq = []
            for i, (h, nb) in enumerate(units):
                eT = emit_scores("d", h, nb, eT_pool, ps_sc)
                if avq:
                    ph, pnb, peT = avq.pop(0)
                    finq.append((ph, pnb, emit_avmm("d", ph, pnb, peT, ps_av)))
                if len(finq) > 1:
                    fin_and_proj("d", finq)
                avq.append((h, nb, eT))
            ph, pnb, peT = avq.pop(0)
            finq.append((ph, pnb, emit_avmm("d", ph, pnb, peT, ps_av)))
            while finq:
                fin_and_proj("d", finq)

            # ---------- Stage E: cond attention + deferred second attention --
            with tc.tile_pool(name="o2pool", bufs=2) as o2p, \
                 tc.tile_pool(name="pdl", bufs=2) as pdl, \
                 tc.tile_pool(name="ps_o2", bufs=1, space="PSUM") as ps_o2:

                def pd_load(mch, pin_ap):
                    # u was pre-normalized; p_d[mch] = pin + bias_d
                    nc.vector.tensor_add(p_d[:, mch, :], pin_ap, bias_bd[:])

                def emit_o2(h, nb, eT, ps_pool, tag):
                    # o2 = (attn_cond @ p_d) per head, normalized by r_c
                    # (K=1 broadcast into rows 64..127); stored transposed
                    ps_o = ps_pool.tile([128, 512], F32, tag=tag)
                    for mch in range(NCH):
                        nc.tensor.matmul(
                            ps_o[0:HD, :], p_d[:, mch, h * HD:(h + 1) * HD],
                            eT[:, mch, :],
                            start=(mch == 0), stop=(mch == NCH - 1))
                    nc.tensor.matmul(
                        ps_o[HD:2 * HD, :], ones_b[:],
                        rT["c"][0:1, h * NB + nb, :], start=True, stop=True)
                    o2T = o2p.tile([HD, 512], F32, tag="o2T")
                    nc.vector.tensor_copy(o2T[:], ps_o[0:HD, :])
                    nc.vector.tensor_mul(o2T[:], o2T[:], ps_o[HD:2 * HD, :])
                    nc.sync.dma_start(
                        out=o_d_t[h, :, nb * 512:(nb + 1) * 512], in_=o2T[:])

                o2q = []
                avq = []
                finq = []
                for i, (h, nb) in enumerate(units):
                    eT = emit_scores("c", h, nb, eT_pool, ps_sc)
                    if avq:
                        ph, pnb, peT = avq.pop(0)
                        finq.append((ph, pnb, emit_avmm("c", ph, pnb, peT, ps_av)))
                    if len(finq) > 1:
                        fin_and_proj("c", finq)
                    if i == LAG:
                        for k in range(2):
                            pin = pdl.tile([128, 4, CW], BF16, tag="p_in", bufs=2,
                                           name=f"pin_{k}")
                            nc.sync.dma_start(
                                out=pin[:],
                                in_=cc_out_d[k].rearrange("(j p) f -> p j f", p=128))
                            for j in range(4):
                                pd_load(k * 4 + j, pin[:, j, :])
                    if i > LAG:
                        uh, unb, ueT = o2q.pop(0)
                        emit_o2(uh, unb, ueT, ps_o2, "o2_ps")
                    avq.append((h, nb, eT))
                    o2q.append((h, nb, eT))
                ph, pnb, peT = avq.pop(0)
                finq.append((ph, pnb, emit_avmm("c", ph, pnb, peT, ps_av)))
                while finq:
                    fin_and_proj("c", finq)

                # o_c stores: half0 (RS_c0 landed during stage E), then the
                # deferred o2 tail (pure PE) overlapping RS_c1, then half1.
                def oc_store(k, ocs):
                    pin = ocs.tile([128, 4, CW], BF16, tag="pc_in")
                    nc.sync.dma_start(
                        out=pin[:],
                        in_=cc_out_c[k].rearrange("(j p) f -> p j f", p=128))
                    ob = ocs.tile([128, 4, CW], F32, tag="oc_out")
                    for j in range(4):
                        nc.vector.tensor_add(ob[:, j, :], pin[:, j, :], bias_bc[:])
                    nc.sync.dma_start(
                        out=o_c_full[k * 512:(k + 1) * 512, :]
                        .rearrange("(j p) f -> p j f", p=128),
                        in_=ob[:])

                with tc.tile_pool(name="oc_pool", bufs=1) as ocs:
                    oc_store(0, ocs)
                    alt = 0
                    while o2q:
                        uh, unb, ueT = o2q.pop(0)
                        emit_o2(uh, unb, ueT,
                                ps_av if alt % 2 else ps_o2,
                                "av_ps" if alt % 2 else "o2_ps")
                        alt += 1
                    oc_store(1, ocs)


def _prep_inputs(x, w_qkv_diff, w_qkv_cond, w_proj_diff, b_proj_diff,
                 w_proj_cond, b_proj_cond):
    import ml_dtypes
    bf = ml_dtypes.bfloat16
    in_maps = []
    for c in range(N_CORES):
        b, hg = divmod(c, 2)
        s = slice(hg * CW, (hg + 1) * CW)
        sk = slice(C + hg * CW, C + (hg + 1) * CW)
        sv = slice(2 * C + hg * CW, 2 * C + (hg + 1) * CW)
        m = {
            "x_b": np.ascontiguousarray(x[b].astype(bf)),
            "wqk_d": np.ascontiguousarray(np.concatenate(
                [w_qkv_diff[:, s] * SCALE, w_qkv_diff[:, sk]], axis=1).astype(bf)),
            "wqk_c": np.ascontiguousarray(np.concatenate(
                [w_qkv_cond[:, s] * SCALE, w_qkv_cond[:, sk]], axis=1).astype(bf)),
            "wv_d": np.ascontiguousarray(w_qkv_diff[:, sv].astype(bf)),
            "wv_c": np.ascontiguousarray(w_qkv_cond[:, sv].astype(bf)),
            "wp_d": np.ascontiguousarray(w_proj_diff[s, :].astype(bf)),
            "wp_c": np.ascontiguousarray(w_proj_cond[s, :].astype(bf)),
            "bias_d": np.ascontiguousarray(b_proj_diff[None, s], np.float32),
            "bias_c": np.ascontiguousarray(b_proj_cond[None, s], np.float32),
        }
        in_maps.append(m)
    return in_maps


def kernel(x, w_qkv_diff, w_qkv_cond, w_proj_diff, b_proj_diff,
           w_proj_cond, b_proj_cond):
    x = np.asarray(x)
    w_qkv_diff = np.asarray(w_qkv_diff)
    w_qkv_cond = np.asarray(w_qkv_cond)
    w_proj_diff = np.asarray(w_proj_diff)
    b_proj_diff = np.asarray(b_proj_diff)
    w_proj_cond = np.asarray(w_proj_cond)
    b_proj_cond = np.asarray(b_proj_cond)

    if "nc" not in _CACHE:
        _CACHE["nc"] = _build()
    nc = _CACHE["nc"]
    in_maps = _prep_inputs(x, w_qkv_diff, w_qkv_cond, w_proj_diff,
                           b_proj_diff, w_proj_cond, b_proj_cond)
    res = run_bass_kernel_spmd(nc, in_maps, list(range(N_CORES))).results

    o_d = np.empty((B, N, C), np.float32)
    o_c = np.empty((B, N, C), np.float32)
    for c in range(N_CORES):
        b, hg = divmod(c, 2)
        odt = np.asarray(res[c]["o_d_t"], np.float32)   # [HPC, HD, N]
        o_d[b][:, hg * CW:(hg + 1) * CW] = odt.transpose(2, 0, 1).reshape(N, CW)
        o_c[b][:, hg * CW:(hg + 1) * CW] = res[c]["o_c_cols"]
    return np.concatenate([o_d, o_c], axis=1)


# revision 40
# speedup vs baseline: 1.0419x; 1.0244x over previous
"""Trainium2 Bass kernel for nn_Attention_cross (dual-branch cross-reuse attention).

Reference computation (B=4, N0=2048, C=768, H=12, hd=64, N=1024):
  x_diff, x_cond = x[:, :N], x[:, N:]
  q,k,v per branch = x @ w_qkv (per-head), attn = softmax(q k^T / sqrt(hd))
  o_d = ((attn_diff @ v_d) @ w_proj_diff + b_d) reused per-head with attn_cond
  o_c = (attn_cond @ v_c) @ w_proj_cond + b_c
  out = concat([o_d, o_c], axis=1)

Sharding: 8 cores = 4 batches x 2 head-groups (6 heads each). The
head-mixing projections are row-sharded with bf16 pair ReduceScatters
(2 chunks per branch) that overlap attention compute. Softmax
normalization is deferred through the linear ops: attention runs on
exp(scores) with row sums riding along as a ones-column in V; the
division is applied at the stores (per-partition scalar broadcasts
plus a small PE expander matmul for column groups). Attention matmul
inputs are bf16 (f32 PSUM accumulation); exp is batched 2 PSUM banks
per ACT instruction; PSUM->SBUF staging copies run on ACT during the
QKV stage where it is otherwise idle.
"""
import numpy as np

import concourse.bass as bass
import concourse.tile as tile
from concourse import bacc, mybir
from concourse.bass_utils import run_bass_kernel_spmd
from concourse.masks import make_identity
from concourse.alu_op_type import AluOpType

F32 = mybir.dt.float32
F32R = mybir.dt.float32r
BF16 = mybir.dt.bfloat16
Exp = mybir.ActivationFunctionType.Exp
Copy = mybir.ActivationFunctionType.Copy

B, N0, C = 4, 2048, 768
H, HD = 12, 64
N = N0 // 2              # 1024 sequence per branch
HPC = H // 2             # 6 heads per core
CW = HPC * HD            # 384 own C-columns/rows
NCH = N // 128           # 8 chunks of 128 along n/m
CCH = C // 128           # 6 chunks of 128 along C
NB = N // 512            # 2 blocks of 512 along n
NH = N // 2              # 512 rows per RS chunk
SCALE = HD ** -0.5
LAG = 5                  # cond units before the first deferred o2 unit

N_CORES = 8
GROUPS = [[0, 1], [2, 3], [4, 5], [6, 7]]

_CACHE = {}


def _build():
    nc = bacc.Bacc("TRN2", target_bir_lowering=False, debug=False,
                   num_devices=N_CORES)

    x_b = nc.dram_tensor("x_b", [N0, C], BF16, kind="ExternalInput").ap()
    wqk_d = nc.dram_tensor("wqk_d", [C, 2 * CW], BF16, kind="ExternalInput").ap()
    wqk_c = nc.dram_tensor("wqk_c", [C, 2 * CW], BF16, kind="ExternalInput").ap()
    wv_d = nc.dram_tensor("wv_d", [C, CW], BF16, kind="ExternalInput").ap()
    wv_c = nc.dram_tensor("wv_c", [C, CW], BF16, kind="ExternalInput").ap()
    wp_d = nc.dram_tensor("wp_d", [CW, C], BF16, kind="ExternalInput").ap()
    wp_c = nc.dram_tensor("wp_c", [CW, C], BF16, kind="ExternalInput").ap()
    bias_d = nc.dram_tensor("bias_d", [1, CW], F32, kind="ExternalInput").ap()
    bias_c = nc.dram_tensor("bias_c", [1, CW], F32, kind="ExternalInput").ap()
    # o_d stored transposed per head: [HPC, HD, N]; host re-layouts.
    o_d_t = nc.dram_tensor("o_d_t", [HPC, HD, N], F32, kind="ExternalOutput").ap()
    o_c_full = nc.dram_tensor("o_c_cols", [N, CW], F32, kind="ExternalOutput").ap()

    # ReduceScatter halves (bf16): input [2, NH, CW] slot-major; each pair
    # member receives its slot reduced: out [NH, CW].
    cc_in_d = [nc.dram_tensor(f"cc_in_d{k}", [2, NH, CW], BF16).ap() for k in range(2)]
    cc_out_d = [nc.dram_tensor(f"cc_out_d{k}", [NH, CW], BF16).ap() for k in range(2)]
    cc_in_c = [nc.dram_tensor(f"cc_in_c{k}", [2, NH, CW], BF16).ap() for k in range(2)]
    cc_out_c = [nc.dram_tensor(f"cc_out_c{k}", [NH, CW], BF16).ap() for k in range(2)]
    eT_dram = nc.dram_tensor("eT_dram", [HPC * NB, 128, NCH * 512], BF16).ap()

    with tile.TileContext(nc) as tc:
        _body(nc, tc, x_b, wqk_d, wqk_c, wv_d, wv_c, wp_d, wp_c,
              bias_d, bias_c, o_d_t, o_c_full,
              cc_in_d, cc_out_d, cc_in_c, cc_out_c, eT_dram)
    nc.compile()
    return nc


def _body(nc, tc, x_b, wqk_d, wqk_c, wv_d, wv_c, wp_d, wp_c,
          bias_d, bias_c, o_d_t, o_c_full,
          cc_in_d, cc_out_d, cc_in_c, cc_out_c, eT_dram):
    from contextlib import ExitStack
    ctx = ExitStack()
    with ctx:
        ctx.enter_context(nc.allow_low_precision(reason="bf16 matmul inputs by design"))
        singles = ctx.enter_context(tc.tile_pool(name="singles", bufs=1))
        ident = singles.tile([128, 128], F32)
        make_identity(nc, ident[:])
        identb = singles.tile([128, 128], BF16)
        nc.vector.tensor_copy(identb[:], ident[:])
        ones_b = singles.tile([1, HD], BF16)
        nc.vector.memset(ones_b[:], 1.0)

        big = ctx.enter_context(tc.tile_pool(name="big", bufs=1))
        qkT = {}    # branch -> [128, 6, N] bf16  rows: [q h0..h5 | k h0..h5]
        v_aug = {}  # branch -> [128, HPC, NCH, 65] bf16 (ones col -> row sums)
        u_t = {}    # branch -> [128, 3, N] bf16 (UNNORMALIZED (expS@v)^T)
        # reciprocal row sums, partition 0 only, free-indexed by unit
        # u = h*NB+nb (partition_broadcast and engine slicing need base 0)
        rT = {}     # branch -> [128, HPC*NB, 512] bf16 (partition 0 used)
        for br in ("d", "c"):
            qkT[br] = big.tile([128, 2 * CW // 128, N], BF16, tag=f"qkT_{br}", name=f"qkT_{br}")
            v_aug[br] = big.tile([128, HPC, NCH, HD + 1], BF16, tag=f"v_{br}", name=f"v_{br}")
            nc.vector.memset(v_aug[br][:, :, :, HD:HD + 1], 1.0)
            u_t[br] = big.tile([128, CW // 128, N], BF16, tag=f"u_{br}", name=f"u_{br}")
            rT[br] = big.tile([128, HPC * NB, 512], BF16, tag=f"rT_{br}", name=f"rT_{br}")
        p_d = big.tile([128, NCH, CW], BF16, tag="p_d")  # normalized+biased proj_diff
        o2acc = big.tile([128, CW // 128, N], BF16, tag="o2acc")  # o2 half-0 partials

        # qkv weights: one consolidated DMA per tensor
        wqk_r = {}
        wv_r = {}
        wp_r = {}
        wsrc = {"d": (wqk_d, wv_d), "c": (wqk_c, wv_c)}
        for br in ("d", "c"):
            wqk_r[br] = big.tile([128, CCH, 2 * CW], BF16, tag=f"wqk_{br}", name=f"wqk_{br}")
            wv_r[br] = big.tile([128, CCH, CW], BF16, tag=f"wv_{br}", name=f"wv_{br}")

        # ---------- Stage A: x load + transpose + QKV for both branches ----
        with tc.tile_pool(name="xt_pool", bufs=1) as xt_pool, \
             tc.tile_pool(name="ab", bufs=3) as ab, \
             tc.tile_pool(name="ps_tp", bufs=1, space="PSUM") as ps_tp, \
             tc.tile_pool(name="ps_ab", bufs=2, space="PSUM") as ps_ab:
            for bi, br in enumerate(("d", "c")):
                half = bi * N
                wqk, wv = wsrc[br]
                xT = xt_pool.tile([128, CCH, N], BF16, tag="xT", bufs=2, name=f"xT_{br}")
                xns = []
                for jg in range(2):
                    xn = ab.tile([128, 4, C], BF16, tag="x_nat", bufs=2)
                    nc.sync.dma_start(
                        out=xn[:],
                        in_=x_b[half + jg * 512: half + (jg + 1) * 512, :]
                        .rearrange("(j p) c -> p j c", p=128))
                    xns.append(xn)
                nc.sync.dma_start(
                    out=wqk_r[br][:],
                    in_=wqk.rearrange("(i p) f -> p i f", p=128))
                nc.sync.dma_start(
                    out=wv_r[br][:],
                    in_=wv.rearrange("(i p) f -> p i f", p=128))
                for jg in range(2):      # n-groups of 4 chunks
                    tp_tiles = {}
                    for ci in range(CCH):
                        tp_tiles[ci] = ps_tp.tile([128, 512], BF16, tag=f"tp{ci}",
                                                  name=f"tp_{br}_{jg}_{ci}")
                    xn = xns[jg]
                    for jj in range(4):
                        for ci in range(CCH):
                            nc.tensor.transpose(
                                tp_tiles[ci][:, jj * 128:(jj + 1) * 128],
                                xn[:, jj, ci * 128:(ci + 1) * 128], identb[:])
                    for ci in range(CCH):
                        nc.scalar.activation(
                            xT[:, ci, jg * 512:(jg + 1) * 512], tp_tiles[ci][:], Copy)
                    # qk/v groups for this half start once its xT cols exist
                    nb = jg
                    for fi in range(2 * CW // 128):
                        ps = ps_ab.tile([128, 512], F32, tag="qk_ps")
                        for ci in range(CCH):
                            nc.tensor.matmul(
                                ps[:],
                                wqk_r[br][:, ci, fi * 128:(fi + 1) * 128],
                                xT[:, ci, nb * 512:(nb + 1) * 512],
                                start=(ci == 0), stop=(ci == CCH - 1))
                        nc.scalar.activation(
                            qkT[br][:, fi, nb * 512:(nb + 1) * 512], ps[:], Copy)
                    for mch in range(jg * 4, jg * 4 + 4):
                        ps = ps_ab.tile([128, 512], F32, tag="qk_ps")
                        for ci in range(CCH):
                            nc.tensor.matmul(
                                ps[:, 0:CW], xT[:, ci, mch * 128:(mch + 1) * 128],
                                wv_r[br][:, ci, :],
                                start=(ci == 0), stop=(ci == CCH - 1))
                        nc.scalar.activation(
                            v_aug[br][:, :, mch, 0:HD],
                            ps[:, 0:CW].rearrange("p (h d) -> p h d", h=HPC), Copy)

        # proj weights + bias: needed from mid stage C on
        for br, wp in (("d", wp_d), ("c", wp_c)):
            wp_r[br] = big.tile([128, CW // 128, C], BF16, tag=f"wp_{br}", name=f"wp_{br}")
            nc.sync.dma_start(out=wp_r[br][:],
                              in_=wp.rearrange("(i p) f -> p i f", p=128))
        bias_bd = big.tile([128, CW], F32, tag="bias_bd")
        nc.sync.dma_start(out=bias_bd[:], in_=bias_d.to_broadcast([128, CW]))
        bias_bc = big.tile([128, CW], F32, tag="bias_bc")
        nc.sync.dma_start(out=bias_bc[:], in_=bias_c.to_broadcast([128, CW]))

        # ---------- attention unit helpers ----------
        def emit_scores(br, h, nb, eT_pool, ps_sc):
            qc, qo = divmod(h * HD, 128)
            kc, ko = divmod(CW + h * HD, 128)
            eT = eT_pool.tile([128, NCH, 512], BF16, tag="eT", bufs=3,
                              name=f"eT_{br}_{h}_{nb}")
            for pr in range(NCH // 2):
                ps = ps_sc.tile([128, 1024], F32, tag="sc_ps")
                for k in range(2):
                    mch = pr * 2 + k
                    nc.tensor.matmul(
                        ps[:, k * 512:(k + 1) * 512],
                        qkT[br][ko:ko + HD, kc, mch * 128:(mch + 1) * 128],
                        qkT[br][qo:qo + HD, qc, nb * 512:(nb + 1) * 512],
                        start=True, stop=True)
                nc.scalar.activation(
                    eT[:, pr * 2:pr * 2 + 2, :].rearrange("p m q -> p (m q)"),
                    ps[:], Exp)
            return eT

        def emit_avmm(br, h, nb, eT, ps_av):
            # AV accumulate (u rows 0..63, row sums at 64) + reciprocal
            ps_u = ps_av.tile([128, 512], F32, tag="av_ps")
            for mch in range(NCH):
                nc.tensor.matmul(
                    ps_u[0:HD + 1, :], v_aug[br][:, h, mch, :], eT[:, mch, :],
                    start=(mch == 0), stop=(mch == NCH - 1))
            nc.vector.reciprocal(rT[br][0:1, h * NB + nb, :], ps_u[HD:HD + 1, :])
            return ps_u

        def emit_fin(br, h, nb, ps_u):
            # broadcast 1/s across 64 rows (K=1 PE matmul into rows 64..127
            # of the same bank), then normalize u into SBUF
            uc, uo = divmod(h * HD, 128)
            nc.tensor.matmul(
                ps_u[HD:2 * HD, :], ones_b[:],
                rT[br][0:1, h * NB + nb, :], start=True, stop=True)
            dst = u_t[br][uo:uo + HD, uc, nb * 512:(nb + 1) * 512]
            nc.vector.tensor_copy(dst, ps_u[0:HD, :])
            nc.vector.tensor_mul(dst, dst, ps_u[HD:2 * HD, :])

        def proj_half(u, wp, cc_in_half, half, pj, ps_pj):
            for nch in range(half * 4, half * 4 + 4):
                st = pj.tile([128, 2, CW], BF16, tag="pj_st")
                for slot in range(2):
                    ps = ps_pj.tile([128, CW], F32, tag="pj_ps")
                    for ci in range(CW // 128):
                        nc.tensor.matmul(
                            ps[:],
                            u[:, ci, nch * 128:(nch + 1) * 128],
                            wp[:, ci, slot * CW:(slot + 1) * CW],
                            start=(ci == 0), stop=(ci == CW // 128 - 1))
                    nc.scalar.activation(st[:, slot, :], ps[:], Copy)
                r0 = (nch - half * 4) * 128
                nc.sync.dma_start(
                    out=cc_in_half[:, r0:r0 + 128, :].rearrange("s p f -> p s f"),
                    in_=st[:])

        with tc.tile_pool(name="eT_pool", bufs=2) as eT_pool, \
             tc.tile_pool(name="pj", bufs=2) as pj, \
             tc.tile_pool(name="ps_sc", bufs=2, space="PSUM") as ps_sc, \
             tc.tile_pool(name="ps_av", bufs=2, space="PSUM") as ps_av, \
             tc.tile_pool(name="ps_pj", bufs=1, space="PSUM") as ps_pj:

            units = [(h, nb) for nb in range(NB) for h in range(HPC)]

            def rs(cc_in_ap, cc_out_ap):
                nc.gpsimd.collective_compute(
                    "ReduceScatter", AluOpType.add, replica_groups=GROUPS,
                    ins=[cc_in_ap], outs=[cc_out_ap])

            cc_in = {"d": cc_in_d, "c": cc_in_c}
            cc_out = {"d": cc_out_d, "c": cc_out_c}

            def fin_and_proj(br, finq):
                fh, fnb, fps = finq.pop(0)
                emit_fin(br, fh, fnb, fps)
                if fh == HPC - 1:
                    proj_half(u_t[br], wp_r[br], cc_in[br][fnb], fnb, pj, ps_pj)
                    rs(cc_in[br][fnb], cc_out[br][fnb])

            # ---------- Stage C: COND attention (spills eT), proj + RS ----
            avq = []
            finq = []
            for i, (h, nb) in enumerate(units):
                eT = emit_scores("c", h, nb, eT_pool, ps_sc)
                nc.sync.dma_start(out=eT_dram[h * NB + nb],
                                  in_=eT[:].rearrange("p m q -> p (m q)"))
                if avq:
                    ph, pnb, peT = avq.pop(0)
                    finq.append((ph, pnb, emit_avmm("c", ph, pnb, peT, ps_av)))
                if len(finq) > 1:
                    fin_and_proj("c", finq)
                avq.append((h, nb, eT))
            ph, pnb, peT = avq.pop(0)
            finq.append((ph, pnb, emit_avmm("c", ph, pnb, peT, ps_av)))
            while finq:
                fin_and_proj("c", finq)

            # ---------- Stage E: DIFF attention; o_c stores overlap --------
            with tc.tile_pool(name="o2pool", bufs=2) as o2p, \
                 tc.tile_pool(name="pdl", bufs=2) as pdl, \
                 tc.tile_pool(name="ocs", bufs=1) as ocs, \
                 tc.tile_pool(name="ps_o2", bufs=1, space="PSUM") as ps_o2:

                def oc_pin(k):
                    pin = ocs.tile([128, 4, CW], BF16, tag="pc_in", bufs=2)
                    nc.sync.dma_start(
                        out=pin[:],
                        in_=cc_out_c[k].rearrange("(j p) f -> p j f", p=128))
                    return pin

                def oc_add(k, pin):
                    ob = ocs.tile([128, 4, CW], F32, tag="oc_out", bufs=2)
                    for j in range(4):
                        nc.vector.tensor_add(ob[:, j, :], pin[:, j, :], bias_bc[:])
                    nc.sync.dma_start(
                        out=o_c_full[k * 512:(k + 1) * 512, :]
                        .rearrange("(j p) f -> p j f", p=128),
                        in_=ob[:])

                def pd_pin(k):
                    pin = pdl.tile([128, 4, CW], BF16, tag="p_in", bufs=2,
                                   name=f"pin_{k}")
                    nc.sync.dma_start(
                        out=pin[:],
                        in_=cc_out_d[k].rearrange("(j p) f -> p j f", p=128))
                    return pin

                def pd_add(k, pin):
                    for j in range(4):
                        nc.vector.tensor_add(p_d[:, k * 4 + j, :], pin[:, j, :],
                                             bias_bd[:])

                pins = {}
                avq = []
                finq = []
                for i, (h, nb) in enumerate(units):
                    eT = emit_scores("d", h, nb, eT_pool, ps_sc)
                    if avq:
                        ph, pnb, peT = avq.pop(0)
                        finq.append((ph, pnb, emit_avmm("d", ph, pnb, peT, ps_av)))
                    if len(finq) > 1:
                        fin_and_proj("d", finq)
                    if i == 0:
                        pins["oc0"] = oc_pin(0)   # after RS_c0/RS_c1 emission
                        pins["oc1"] = oc_pin(1)
                    if i == 3:
                        oc_add(0, pins["oc0"])    # RS_c0 landed
                    if i == 9:
                        oc_add(1, pins["oc1"])    # RS_c1 landed; before RS_d0's
                    if i == 11:
                        pins["pd0"] = pd_pin(0)   # after RS_d0, before RS_d1
                    avq.append((h, nb, eT))
                ph, pnb, peT = avq.pop(0)
                finq.append((ph, pnb, emit_avmm("d", ph, pnb, peT, ps_av)))
                while finq:
                    fin_and_proj("d", finq)

                # -------- drain: second attention in key-halves ------------
                def o2_half(h, nb, eTr, half, ps_pool, tag):
                    uc, uo = divmod(h * HD, 128)
                    ps_o = ps_pool.tile([128, 512], F32, tag=tag)
                    for j in range(4):
                        mch = half * 4 + j
                        nc.tensor.matmul(
                            ps_o[0:HD, :], p_d[:, mch, h * HD:(h + 1) * HD],
                            eTr[:, j, :],
                            start=(j == 0), stop=(j == 3))
                    if half == 0:
                        nc.scalar.activation(
                            o2acc[uo:uo + HD, uc, nb * 512:(nb + 1) * 512],
                            ps_o[0:HD, :], Copy)
                    else:
                        nc.tensor.matmul(
                            ps_o[HD:2 * HD, :], ones_b[:],
                            rT["c"][0:1, h * NB + nb, :], start=True, stop=True)
                        o2T = o2p.tile([HD, 512], F32, tag="o2T")
                        nc.vector.tensor_add(
                            o2T[:],
                            o2acc[uo:uo + HD, uc, nb * 512:(nb + 1) * 512],
                            ps_o[0:HD, :])
                        nc.vector.tensor_mul(o2T[:], o2T[:], ps_o[HD:2 * HD, :])
                        nc.sync.dma_start(
                            out=o_d_t[h, :, nb * 512:(nb + 1) * 512], in_=o2T[:])

                with tc.tile_pool(name="eTr_pool", bufs=4) as eTr_pool:
                    def reload(h, nb, half):
                        eTr = eTr_pool.tile([128, 4, 512], BF16, tag="eTr")
                        nc.sync.dma_start(
                            out=eTr[:],
                            in_=eT_dram[h * NB + nb][:, half * 2048:(half + 1) * 2048]
                            .rearrange("p (m q) -> p m q", m=4))
                        return eTr

                    pd_add(0, pins["pd0"])
                    alt = 0
                    for h, nb in units:
                        eTr = reload(h, nb, 0)
                        o2_half(h, nb, eTr, 0,
                                ps_av if alt % 2 else ps_o2,
                                "av_ps" if alt % 2 else "o2_ps")
                        alt += 1
                    pin1 = pd_pin(1)             # after RS_d1 (last collective)
                    pd_add(1, pin1)
                    for h, nb in units:
                        eTr = reload(h, nb, 1)
                        o2_half(h, nb, eTr, 1,
                                ps_av if alt % 2 else ps_o2,
                                "av_ps" if alt % 2 else "o2_ps")
                        alt += 1


def _prep_inputs(x, w_qkv_diff, w_qkv_cond, w_proj_diff, b_proj_diff,
                 w_proj_cond, b_proj_cond):
    import ml_dtypes
    bf = ml_dtypes.bfloat16
    in_maps = []
    for c in range(N_CORES):
        b, hg = divmod(c, 2)
        s = slice(hg * CW, (hg + 1) * CW)
        sk = slice(C + hg * CW, C + (hg + 1) * CW)
        sv = slice(2 * C + hg * CW, 2 * C + (hg + 1) * CW)
        m = {
            "x_b": np.ascontiguousarray(x[b].astype(bf)),
            "wqk_d": np.ascontiguousarray(np.concatenate(
                [w_qkv_diff[:, s] * SCALE, w_qkv_diff[:, sk]], axis=1).astype(bf)),
            "wqk_c": np.ascontiguousarray(np.concatenate(
                [w_qkv_cond[:, s] * SCALE, w_qkv_cond[:, sk]], axis=1).astype(bf)),
            "wv_d": np.ascontiguousarray(w_qkv_diff[:, sv].astype(bf)),
            "wv_c": np.ascontiguousarray(w_qkv_cond[:, sv].astype(bf)),
            "wp_d": np.ascontiguousarray(w_proj_diff[s, :].astype(bf)),
            "wp_c": np.ascontiguousarray(w_proj_cond[s, :].astype(bf)),
            "bias_d": np.ascontiguousarray(b_proj_diff[None, s], np.float32),
            "bias_c": np.ascontiguousarray(b_proj_cond[None, s], np.float32),
        }
        in_maps.append(m)
    return in_maps


def kernel(x, w_qkv_diff, w_qkv_cond, w_proj_diff, b_proj_diff,
           w_proj_cond, b_proj_cond):
    x = np.asarray(x)
    w_qkv_diff = np.asarray(w_qkv_diff)
    w_qkv_cond = np.asarray(w_qkv_cond)
    w_proj_diff = np.asarray(w_proj_diff)
    b_proj_diff = np.asarray(b_proj_diff)
    w_proj_cond = np.asarray(w_proj_cond)
    b_proj_cond = np.asarray(b_proj_cond)

    if "nc" not in _CACHE:
        _CACHE["nc"] = _build()
    nc = _CACHE["nc"]
    in_maps = _prep_inputs(x, w_qkv_diff, w_qkv_cond, w_proj_diff,
                           b_proj_diff, w_proj_cond, b_proj_cond)
    res = run_bass_kernel_spmd(nc, in_maps, list(range(N_CORES))).results

    o_d = np.empty((B, N, C), np.float32)
    o_c = np.empty((B, N, C), np.float32)
    for c in range(N_CORES):
        b, hg = divmod(c, 2)
        odt = np.asarray(res[c]["o_d_t"], np.float32)   # [HPC, HD, N]
        o_d[b][:, hg * CW:(hg + 1) * CW] = odt.transpose(2, 0, 1).reshape(N, CW)
        o_c[b][:, hg * CW:(hg + 1) * CW] = res[c]["o_c_cols"]
    return np.concatenate([o_d, o_c], axis=1)
